# revision 15
# baseline (speedup 1.0000x reference)
"""Causal multi-head self-attention (B=4, S=2048, D=1024, H=16) on 8 TRN2
NeuronCores.

Sharding: core c = (batch b = c//2, head-half = c%2). Each core computes, for
its batch and its 8 heads: QKV projections (+RoPE), causal softmax attention,
and a row-sharded output projection. The host sums the two partial y's per
batch.

v7 design (on top of v5's fp16 + packed-causal + woven schedule):
  - Zero tiling-mode switches: kT is stored as TWO zero-padded tiles
    (kT0p: head0 dims in partitions 0:64, zeros in 64:128; kT1p mirrored),
    so the score matmuls contract over K=128 and every matmul in the kernel
    runs in the default 128x128 array mode. v5/v6's K=64 row-tiled score
    matmuls overlapped per-pair but paid a PE drain (~110ns) on every
    64<->128 mode transition (~300 of them, ~35us).
  - RoPE: the +-32 partition swap runs as two SBUF->SBUF DMAs (affine
    2-block partition patterns) on otherwise-idle DMA queues, replacing 4
    DVE strip-muls; the sin multiply is then one full-width fp16 op.
  - v-slice PSUM evacuations moved to ScalarE (idle during hc=0's
    attention); projection fillers distributed per-2-i-blocks; pair-3 runs
    j in [3,2,1,0]; final wo group ping-pongs over 4 PSUM banks.
  - Startup: first x/wq chunks DMA'd first so the first matmul issues at
    ~10us (the ~7us engine-barrier preamble + DMA dispatch rate bound it).
"""

import numpy as np

B, S, D = 4, 2048, 1024
NUM_HEADS = 16
THETA = 10000.0
DH = 64
N_CORES = 8
P = 128

_CACHE = {}


def build_nc():
    """Build the single-core SPMD Bass program (identical on all 8 cores)."""
    import concourse.mybir as mybir
    import concourse.tile as tile
    from concourse import bacc
    from concourse.bass import ts

    F16 = mybir.dt.float16
    F32 = mybir.dt.float32
    Act = mybir.ActivationFunctionType

    nc = bacc.Bacc(trn_type="TRN2")
    xT_d = nc.dram_tensor("xT", [D, S], F16, kind="ExternalInput")
    wqT_d = nc.dram_tensor("wqT", [D, 512], F16, kind="ExternalInput")
    wkT_d = nc.dram_tensor("wkT", [D, 512], F16, kind="ExternalInput")
    wvT_d = nc.dram_tensor("wvT", [D, 512], F16, kind="ExternalInput")
    woT_d = nc.dram_tensor("woT", [512, D], F16, kind="ExternalInput")
    cosT_d = nc.dram_tensor("cosT", [P, S], F16, kind="ExternalInput")
    sinT_d = nc.dram_tensor("sinT", [P, S], F16, kind="ExternalInput")
    tri_d = nc.dram_tensor("tri", [P, P], F16, kind="ExternalInput")
    y_d = nc.dram_tensor("y", [S, D], F16, kind="ExternalOutput")

    xT3 = xT_d.ap().rearrange("(kc p) s -> p kc s", p=P)     # [128, 8, 2048]
    wq3 = wqT_d.ap().rearrange("(kc p) j -> p kc j", p=P)    # [128, 8, 512]
    wk3 = wkT_d.ap().rearrange("(kc p) j -> p kc j", p=P)
    wv3 = wvT_d.ap().rearrange("(kc p) j -> p kc j", p=P)
    wo3 = woT_d.ap().rearrange("(jc p) i -> p jc i", p=P)    # [128, 4, 1024]
    y_ap = y_d.ap()

    with tile.TileContext(nc) as tc:
        with (
            tc.tile_pool(name="pers", bufs=1) as pers,
            tc.tile_pool(name="w1", bufs=1) as w1,
            tc.tile_pool(name="x1", bufs=4) as x1,
            tc.tile_pool(name="sq", bufs=4) as sq,
            tc.tile_pool(name="tmp1", bufs=2) as tmp1,
            tc.tile_pool(name="swp", bufs=2) as swp,
            tc.tile_pool(name="ptp", bufs=4) as ptp,
            tc.tile_pool(name="rcp", bufs=4) as rcp,
            tc.tile_pool(name="rcd", bufs=2) as rcd,
            tc.tile_pool(name="rbp", bufs=3) as rbp,
            tc.tile_pool(name="wo", bufs=1) as wo,
            tc.tile_pool(name="ysb", bufs=3) as ysb,
            tc.tile_pool(name="drm", bufs=2, space="DRAM") as drm,
        ):
            qT = pers.tile([P, 4, S], F16)
            kT0p = pers.tile([P, 4, S], F16)
            kT1p = pers.tile([P, 4, S], F16)
            vA = pers.tile([P, 16, 8, 65], F16)
            outT = pers.tile([P, 4, S], F16)
            cosb = pers.tile([P, S], F16)
            sinb = pers.tile([P, S], F16)
            trib = pers.tile([P, P], F16)
            dummy = pers.tile([1, 1], F16)
            wq_s = w1.tile([P, 8, 512], F16)
            wk_s = w1.tile([P, 8, 512], F16)
            wv_s = w1.tile([P, 8, 512], F16)
            wo_s = wo.tile([P, 4, D], F16)
            xs_tiles = []
            for _sl in range(4):
                xs_t = x1.tile([P, 8, 512], F16, tag="xs")
                xs_tiles.append(xs_t)

            # DMA order: unblock the first q-projection matmul (xs0-kc0 +
            # wq-kc0) in 2 dispatches, then stream the rest in need order.
            nc.sync.dma_start(xs_tiles[0][:, 0:1, :], xT3[:, 0:1, ts(0, 512)])
            nc.sync.dma_start(wq_s[:, 0:1, :], wq3[:, 0:1, :])
            nc.sync.dma_start(xs_tiles[0][:, 1:4, :], xT3[:, 1:4, ts(0, 512)])
            nc.sync.dma_start(wq_s[:, 1:4, :], wq3[:, 1:4, :])
            nc.sync.dma_start(xs_tiles[0][:, 4:8, :], xT3[:, 4:8, ts(0, 512)])
            nc.sync.dma_start(wq_s[:, 4:8, :], wq3[:, 4:8, :])
            for kh in range(2):
                k4 = slice(4 * kh, 4 * kh + 4)
                nc.sync.dma_start(wk_s[:, k4, :], wk3[:, k4, :])
            nc.sync.dma_start(cosb[:], cosT_d.ap())
            nc.sync.dma_start(sinb[:], sinT_d.ap())
            nc.sync.dma_start(trib[:], tri_d.ap())
            for kh in range(2):
                k4 = slice(4 * kh, 4 * kh + 4)
                nc.sync.dma_start(wv_s[:, k4, :], wv3[:, k4, :])
            for sl in (1, 2, 3):
                for kh in range(4):
                    k2 = slice(2 * kh, 2 * kh + 2)
                    nc.sync.dma_start(
                        xs_tiles[sl][:, k2, :], xT3[:, k2, ts(sl, 512)]
                    )
            nc.sync.dma_start(wo_s[:], wo3)
            nc.vector.memset(vA[:, :, :, 64:65], 1.0)
            nc.vector.memset(dummy[:], 0.0)
            # zero the dead halves of the padded k tiles once; the score
            # matmuls then contract over all 128 partitions safely
            nc.vector.memset(kT0p[64:P, :, :], 0.0)
            nc.vector.memset(kT1p[0:64, :, :], 0.0)
            # dummy 1-element exp: forces the ACT Exp table load during the
            # prologue instead of on the first real exp chain
            wrm = rcd.tile([1, 1], F16, tag="wrm")
            nc.scalar.activation(wrm[:], dummy[:], Act.Exp)

            # PSUM: psP (proj/v) 2 banks + psB (sc) 4 + psC (pa) 2 = 8
            _psP_cm = tc.tile_pool(name="psP", bufs=2, space="PSUM")
            _psB_cm = tc.tile_pool(name="psB", bufs=2, space="PSUM")
            _psC_cm = tc.tile_pool(name="psC", bufs=2, space="PSUM")
            psP = _psP_cm.__enter__()
            psB = _psB_cm.__enter__()
            psC = _psC_cm.__enter__()

            def rope(pq, dsts, sls):
                # fp16 on-chip rope. The +-32 partition swap runs as two
                # SBUF->SBUF DMAs (each an affine 2-block partition
                # pattern) on otherwise-idle DMA queues; DVE then does one
                # full-width signed-sin multiply instead of 4 strip-muls.
                pq_s = sq.tile([P, 512], F16, tag="pqs")
                nc.vector.tensor_copy(pq_s[:], pq[:])
                tA = tmp1.tile([P, 512], F16, tag="tA")
                nc.vector.tensor_mul(tA[:], pq_s[:], cosb[:, sls])
                tBs = tmp1.tile([P, 512], F16, tag="tBs")
                for hb in (0, 64):
                    nc.vector.tensor_mul(
                        tBs[hb : hb + 32, :],
                        pq_s[hb + 32 : hb + 64, :],
                        sinb[hb + 32 : hb + 64, sls],
                    )
                    nc.vector.tensor_mul(
                        tBs[hb + 32 : hb + 64, :],
                        pq_s[hb : hb + 32, :],
                        sinb[hb : hb + 32, sls],
                    )
                for dst, lo, hi in dsts:
                    nc.vector.tensor_add(dst, tA[lo:hi, :], tBs[lo:hi, :])

            def q_dsts(hc, sls):
                return [(qT[:, hc, sls], 0, P)]

            def k_dsts(hc, sls):
                return [(kT0p[0:64, hc, sls], 0, 64), (kT1p[64:P, hc, sls], 64, P)]

            def proj_one(hc, sl, w_s, dst_fn):
                # one q-or-k projection blob (8 matmuls) + inline rope
                sls = ts(sl, 512)
                xs = xs_tiles[sl]
                pq = psP.tile([P, 512], F32, tag="pp")
                for kc in range(8):
                    nc.tensor.matmul(
                        pq[:], w_s[:, kc, ts(hc, P)], xs[:, kc, :],
                        start=(kc == 0), stop=(kc == 7),
                    )
                rope(pq, dst_fn(hc, sls), sls)

            def proj_halves(hc, sl, w_s, dst_fn):
                # the same blob split into two 4-matmul filler units; the
                # rope is emitted inline right after the second half so DVE
                # work spreads through the block instead of bursting at its
                # end
                sls = ts(sl, 512)
                xs = xs_tiles[sl]
                state = {}

                def first():
                    state["pq"] = psP.tile(
                        [P, 512], F32, tag="pp", name="pqh"
                    )
                    for kc in range(4):
                        nc.tensor.matmul(
                            state["pq"][:], w_s[:, kc, ts(hc, P)],
                            xs[:, kc, :], start=(kc == 0), stop=False,
                        )

                def second():
                    pq = state["pq"]
                    for kc in range(4, 8):
                        nc.tensor.matmul(
                            pq[:], w_s[:, kc, ts(hc, P)], xs[:, kc, :],
                            start=False, stop=(kc == 7),
                        )
                    rope(pq, dst_fn(hc, sls), sls)

                return [first, second]

            def v_unit(sl, t4):
                # one t4 block of a v-slice (8 matmuls); PSUM evacuation on
                # ScalarE (idle during hc=0's attention)
                xs = xs_tiles[sl]
                pv = psP.tile([P, 512], F32, tag="pp")
                for kc in range(8):
                    nc.tensor.matmul(
                        pv[:], xs[:, kc, ts(t4, P)], wv_s[:, kc, :],
                        start=(kc == 0), stop=(kc == 7),
                    )
                nc.scalar.copy(
                    vA[:, sl * 4 + t4, :, 0:64],
                    pv.rearrange("p (h c) -> p h c", h=8),
                )

            def sc_mms(hc, j, i):
                # packed causal layout: head0 cols [w0:512] (q -> col q),
                # head1 cols [512:1024-w0] (q -> col 512+q-w0). K=128 via
                # the zero-padded kT tiles: no tiling-mode switch.
                w0 = max(i - 4 * j, 0) * P
                sc = psB.tile([P, 1024], F32, tag="sc")
                nc.tensor.matmul(
                    sc[:, w0:512], kT0p[:, hc, ts(i, P)],
                    qT[:, hc, j * 512 + w0 : (j + 1) * 512],
                    start=True, stop=True,
                )
                nc.tensor.matmul(
                    sc[:, 512 : 1024 - w0], kT1p[:, hc, ts(i, P)],
                    qT[:, hc, j * 512 + w0 : (j + 1) * 512],
                    start=True, stop=True,
                )
                return sc, w0

            def exp_pa(hc, j, i, sc, w0, pa0, pa1, last):
                # one contiguous exp over both heads' causal region; the
                # diagonal 128-blocks are zeroed post-exp by fp16 tri-mask
                # multiplies (the masked region of sc holds stale-but-finite
                # PSUM values, so exp is safe).
                diag = i - 4 * j >= 0
                pt = ptp.tile([P, 1024], F16, tag="pt")
                nc.scalar.activation(
                    pt[:, 0 : 1024 - 2 * w0], sc[:, w0 : 1024 - w0], Act.Exp
                )
                if diag:
                    nc.vector.tensor_mul(pt[:, 0:P], pt[:, 0:P], trib[:])
                    nc.vector.tensor_mul(
                        pt[:, 512 - w0 : 640 - w0],
                        pt[:, 512 - w0 : 640 - w0], trib[:],
                    )
                nc.tensor.matmul(
                    pa0[:, w0:512], vA[:, i, 2 * hc, :],
                    pt[:, 0 : 512 - w0],
                    start=(i == 0), stop=(i == last),
                )
                nc.tensor.matmul(
                    pa1[:, w0:512], vA[:, i, 2 * hc + 1, :],
                    pt[:, 512 - w0 : 1024 - 2 * w0],
                    start=(i == 0), stop=(i == last),
                )

            den_tiles = {}

            def attention_block(hc, j, den_d, rbase, fillers=None):
                # rolling pipeline: sc(i+1) prefetches while exp(i) runs;
                # projection fillers emitted every other i-block so ScalarE's
                # exp stream overlaps projection matmuls
                pa0 = psC.tile([65, 512], F32, tag="pa")
                pa1 = psC.tile([65, 512], F32, tag="pa")
                last = 4 * j + 3
                fb = list(fillers or [])
                sc_prev = sc_mms(hc, j, 0)
                for i in range(last + 1):
                    sc_next = sc_mms(hc, j, i + 1) if i < last else None
                    if fb:
                        f = fb.pop(0)
                        if f is not None:
                            f()
                    exp_pa(hc, j, i, *sc_prev, pa0, pa1, last)
                    sc_prev = sc_next
                srows = []
                for h01, pa in ((0, pa0), (1, pa1)):
                    srow = rcp.tile([1, 512], F32, tag="srow")
                    nc.vector.tensor_copy(srow[:], pa[64:65, 0:512])
                    if den_d is not None:
                        nc.sync.dma_start(
                            den_d[rbase + h01 : rbase + h01 + 1, :], srow[:]
                        )
                    srows.append(srow)
                    nc.vector.tensor_copy(
                        outT[h01 * 64 : h01 * 64 + 64, hc, ts(j, 512)],
                        pa[0:64, 0:512],
                    )
                return srows

            def epilogue_pair(hc):
                den_sb = rcd.tile([8, 512], F32, tag="densb")
                nc.sync.dma_start(den_sb[:], den_tiles[hc][:])
                rec32 = rcd.tile([8, 512], F32, tag="rec32")
                nc.vector.reciprocal_approx_fast(rec32[:], den_sb[:])
                rec8 = rcd.tile([8, 512], F16, tag="rec8")
                with nc.allow_low_precision(reason="fp16 softmax denom"):
                    nc.vector.tensor_copy(rec8[:], rec32[:])
                rec_d = drm.tile([8, 512], F16, tag="recd")
                nc.sync.dma_start(rec_d[:], rec8[:])
                for j in range(4):
                    rb = rbp.tile([P, 512], F16, tag="rb")
                    for h01 in range(2):
                        r = 2 * j + h01
                        nc.sync.dma_start(
                            rb[h01 * 64 : h01 * 64 + 64, :],
                            rec_d[r : r + 1, :].broadcast_to((64, 512)),
                        )
                    nc.vector.tensor_mul(
                        outT[:, hc, ts(j, 512)], outT[:, hc, ts(j, 512)], rb[:]
                    )

            # ---- output projection unit: one 128-query block of the
            # output projection (8 matmuls + evacuation + store); runs as a
            # filler inside pair-3's attention, borrowing psP (and psC for
            # the final group) ----
            def p3_unit(st, pool):
                tag = "pa" if pool is psC else "pp"
                py0 = pool.tile([P, 512], F32, tag=tag)
                py1 = pool.tile([P, 512], F32, tag=tag)
                # jc-outer so the two halves share each outT stationary
                for jc in range(4):
                    nc.tensor.matmul(
                        py0[:], outT[:, jc, ts(st, P)], wo_s[:, jc, 0:512],
                        start=(jc == 0), stop=(jc == 3),
                    )
                    nc.tensor.matmul(
                        py1[:], outT[:, jc, ts(st, P)], wo_s[:, jc, 512:D],
                        start=(jc == 0), stop=(jc == 3),
                    )
                yo0 = ysb.tile([P, 512], F16, tag="yo0")
                yo1 = ysb.tile([P, 512], F16, tag="yo1")
                with nc.allow_low_precision(reason="fp16 partial y"):
                    nc.vector.tensor_copy(yo0[:], py0[:])
                    nc.vector.tensor_copy(yo1[:], py1[:])
                nc.sync.dma_start(y_ap[ts(st, P), 0:512], yo0[:])
                nc.sync.dma_start(y_ap[ts(st, P), 512:D], yo1[:])

            def epilogue_j3(j, srows):
                # per-j denominator chain for the last pair: direct
                # reciprocal on the partition-0 srow tiles, one DRAM hop
                # for the partition-broadcast.
                rec_d2 = drm.tile([2, 512], F32, tag="recd2")
                for h01 in range(2):
                    r32 = rcd.tile([1, 512], F32, tag="r32b")
                    nc.vector.reciprocal_approx_fast(r32[:], srows[h01][:])
                    nc.sync.dma_start(rec_d2[h01 : h01 + 1, :], r32[:])
                rb = rbp.tile([P, 512], F32, tag="rb32")
                for h01 in range(2):
                    nc.sync.dma_start(
                        rb[h01 * 64 : h01 * 64 + 64, :],
                        rec_d2[h01 : h01 + 1, :].broadcast_to((64, 512)),
                    )
                nc.vector.tensor_mul(
                    outT[:, 3, ts(j, 512)], outT[:, 3, ts(j, 512)], rb[:]
                )

            # ---- woven schedule ----
            # mini-prologue: q/k/v for slice 0 of pair 0 only; everything
            # else weaves into the attention blocks below, one ~0.85us
            # filler unit per i-block so the PE never idles on the exp
            # chain and DVE/ACT work stays spread out.
            proj_one(0, 0, wq_s, q_dsts)
            proj_one(0, 0, wk_s, k_dsts)
            for t4 in range(4):
                v_unit(0, t4)

            def qh(hc, sl):
                return proj_halves(hc, sl, wq_s, q_dsts)

            def kh(hc, sl):
                return proj_halves(hc, sl, wk_s, k_dsts)

            def vu(sl, t4):
                return lambda: v_unit(sl, t4)

            def spread(units, slots):
                out = [None] * slots
                step = max(1, slots // len(units))
                for n, u in enumerate(units):
                    out[n * step] = u
                return out

            fillers = {
                (0, 0): qh(0, 1) + kh(0, 1),
                (0, 1): [vu(1, 0), vu(1, 1), vu(1, 2), vu(1, 3)]
                        + qh(0, 2) + kh(0, 2),
                (0, 2): [vu(2, 0), vu(2, 1), vu(2, 2), vu(2, 3)]
                        + qh(0, 3) + kh(0, 3) + qh(1, 0) + kh(1, 0),
                (0, 3): [vu(3, 0), vu(3, 1), vu(3, 2), vu(3, 3)]
                        + qh(1, 1) + kh(1, 1) + qh(1, 2) + kh(1, 2)
                        + qh(1, 3) + kh(1, 3),
            }
            for src_hc, dst_hc in ((1, 2), (2, 3)):
                for j in range(4):
                    units = qh(dst_hc, j) + kh(dst_hc, j)
                    fillers[(src_hc, j)] = spread(units, 4 * j + 4)

            for hc in range(3):
                den_d = drm.tile([8, 512], F32, tag="dend")
                den_tiles[hc] = den_d
                for j in range(4):
                    attention_block(hc, j, den_d, 2 * j,
                                    fillers=fillers[(hc, j)])
                if hc >= 1:
                    epilogue_pair(hc - 1)
            # pair 3: j in [3,2,1,0] so the tail attention block is the
            # smallest; p3 wo-projection st-units weave into the NEXT
            # attention block's filler slots so their denominator chains
            # hide under it.
            def p3u(st, pool):
                return lambda: p3_unit(st, pool)

            p3f = {
                3: [],
                2: spread([p3u(12 + n, psP) for n in range(4)], 12),
                1: spread([p3u(8 + n, psP) for n in range(4)], 8),
                0: [p3u(4 + n, psP) for n in range(4)],
            }
            j_order = [3, 2, 1, 0]
            for idx, j in enumerate(j_order):
                srows = attention_block(3, j, None, 0, fillers=p3f[j])
                if idx == 0:
                    epilogue_pair(2)
                epilogue_j3(j, srows)
            for n in range(4):
                p3_unit(n, psP if n % 2 == 0 else psC)

            _psC_cm.__exit__(None, None, None)
            _psB_cm.__exit__(None, None, None)
            _psP_cm.__exit__(None, None, None)

    nc.compile()
    return nc


def prep_core_inputs(x, token_ids, Wq, Wk, Wv, Wo, core):
    b, half = divmod(core, 2)
    rows = []
    for h in range(half * 8, half * 8 + 8):
        base = h * DH
        rows.extend(base + np.arange(0, DH, 2))
        rows.extend(base + np.arange(1, DH, 2))
    rows = np.asarray(rows)
    cols = np.arange(half * 512, half * 512 + 512)

    f16 = np.float16
    f32 = np.float32
    inv = THETA ** (-np.arange(0, DH, 2, dtype=np.float64) / DH)
    ang = np.asarray(token_ids, dtype=np.float64)[None, :] * inv[:, None]
    cosT = np.tile(np.cos(ang), (4, 1)).astype(f16)
    # signed sin table, source-indexed: the swap-muls read pq_s and sinT at
    # the SOURCE partitions (rows 0:32 = +sin, 32:64 = -sin, tiled)
    sin_block = np.concatenate([np.sin(ang), -np.sin(ang)], axis=0)
    sinT = np.tile(sin_block, (2, 1)).astype(f16)
    tri = (np.arange(P)[:, None] <= np.arange(P)[None, :]).astype(f16)
    return {
        "xT": np.ascontiguousarray(np.asarray(x[b], f32).T.astype(f16)),
        "wqT": np.ascontiguousarray((np.asarray(Wq, f32)[rows] * 0.125).T.astype(f16)),
        "wkT": np.ascontiguousarray(np.asarray(Wk, f32)[rows].T.astype(f16)),
        "wvT": np.ascontiguousarray(np.asarray(Wv, f32)[cols].T.astype(f16)),
        "woT": np.ascontiguousarray(np.asarray(Wo, f32)[:, cols].T.astype(f16)),
        "cosT": cosT,
        "sinT": sinT,
        "tri": tri,
    }


def get_nc():
    if "nc" not in _CACHE:
        _CACHE["nc"] = build_nc()
    return _CACHE["nc"]


def run_cores(in_maps, trace=False):
    from concourse.bass_utils import run_bass_kernel_spmd

    return run_bass_kernel_spmd(
        get_nc(), in_maps, core_ids=list(range(N_CORES)), trace=trace
    )


def combine(res):
    y = np.empty((B, S, D), np.float32)
    for b in range(B):
        y[b] = res.results[2 * b]["y"].astype(np.float32) + res.results[
            2 * b + 1
        ]["y"].astype(np.float32)
    return y


def kernel(x, token_ids, Wq, Wk, Wv, Wo):
    in_maps = [
        prep_core_inputs(x, token_ids, Wq, Wk, Wv, Wo, c) for c in range(N_CORES)
    ]
    res = run_cores(in_maps)
    return combine(res)


# revision 16
# speedup vs baseline: 1.1335x; 1.1335x over previous
"""Causal multi-head self-attention (B=4, S=2048, D=1024, H=16) on 8 TRN2
NeuronCores.

Sharding: core c = (batch b = c//2, head-half = c%2). Each core computes, for
its batch and its 8 heads: QKV projections (+RoPE), causal softmax attention,
and a row-sharded output projection. The host sums the two partial y's per
batch.

v7 design (on top of v5's fp16 + packed-causal + woven schedule):
  - Zero tiling-mode switches: kT is stored as TWO zero-padded tiles
    (kT0p: head0 dims in partitions 0:64, zeros in 64:128; kT1p mirrored),
    so the score matmuls contract over K=128 and every matmul in the kernel
    runs in the default 128x128 array mode. v5/v6's K=64 row-tiled score
    matmuls overlapped per-pair but paid a PE drain (~110ns) on every
    64<->128 mode transition (~300 of them, ~35us).
  - RoPE: the +-32 partition swap runs as two SBUF->SBUF DMAs (affine
    2-block partition patterns) on otherwise-idle DMA queues, replacing 4
    DVE strip-muls; the sin multiply is then one full-width fp16 op.
  - v-slice PSUM evacuations moved to ScalarE (idle during hc=0's
    attention); projection fillers distributed per-2-i-blocks; pair-3 runs
    j in [3,2,1,0]; final wo group ping-pongs over 4 PSUM banks.
  - Startup: first x/wq chunks DMA'd first so the first matmul issues at
    ~10us (the ~7us engine-barrier preamble + DMA dispatch rate bound it).
"""

import numpy as np

B, S, D = 4, 2048, 1024
NUM_HEADS = 16
THETA = 10000.0
DH = 64
N_CORES = 8
P = 128

_CACHE = {}


def build_nc():
    """Build the single-core SPMD Bass program (identical on all 8 cores)."""
    import concourse.mybir as mybir
    import concourse.tile as tile
    from concourse import bacc
    from concourse.bass import ts

    F16 = mybir.dt.float16
    F32 = mybir.dt.float32
    Act = mybir.ActivationFunctionType

    nc = bacc.Bacc(trn_type="TRN2")
    xT_d = nc.dram_tensor("xT", [D, S], F16, kind="ExternalInput")
    wqT_d = nc.dram_tensor("wqT", [D, 512], F16, kind="ExternalInput")
    wkT_d = nc.dram_tensor("wkT", [D, 512], F16, kind="ExternalInput")
    wvT_d = nc.dram_tensor("wvT", [D, 512], F16, kind="ExternalInput")
    woT_d = nc.dram_tensor("woT", [512, D], F16, kind="ExternalInput")
    cosT_d = nc.dram_tensor("cosT", [P, S], F16, kind="ExternalInput")
    sinT_d = nc.dram_tensor("sinT", [P, S], F16, kind="ExternalInput")
    tri_d = nc.dram_tensor("tri", [P, P], F16, kind="ExternalInput")
    y_d = nc.dram_tensor("y", [S, D], F16, kind="ExternalOutput")

    xT3 = xT_d.ap().rearrange("(kc p) s -> p kc s", p=P)     # [128, 8, 2048]
    wq3 = wqT_d.ap().rearrange("(kc p) j -> p kc j", p=P)    # [128, 8, 512]
    wk3 = wkT_d.ap().rearrange("(kc p) j -> p kc j", p=P)
    wv3 = wvT_d.ap().rearrange("(kc p) j -> p kc j", p=P)
    wo3 = woT_d.ap().rearrange("(jc p) i -> p jc i", p=P)    # [128, 4, 1024]
    y_ap = y_d.ap()

    with tile.TileContext(nc) as tc:
        with (
            tc.tile_pool(name="pers", bufs=1) as pers,
            tc.tile_pool(name="w1", bufs=1) as w1,
            tc.tile_pool(name="x1", bufs=4) as x1,
            tc.tile_pool(name="sq", bufs=4) as sq,
            tc.tile_pool(name="tmp1", bufs=2) as tmp1,
            tc.tile_pool(name="swp", bufs=2) as swp,
            tc.tile_pool(name="ptp", bufs=4) as ptp,
            tc.tile_pool(name="rcp", bufs=4) as rcp,
            tc.tile_pool(name="rcd", bufs=2) as rcd,
            tc.tile_pool(name="rbp", bufs=3) as rbp,
            tc.tile_pool(name="wo", bufs=1) as wo,
            tc.tile_pool(name="ysb", bufs=3) as ysb,
            tc.tile_pool(name="drm", bufs=2, space="DRAM") as drm,
        ):
            qT = pers.tile([P, 4, S], F16)
            kT0p = pers.tile([P, 4, S], F16)
            kT1p = pers.tile([P, 4, S], F16)
            vA = pers.tile([P, 16, 8, 65], F16)
            outT = pers.tile([P, 4, S], F16)
            cosb = pers.tile([P, S], F16)
            sinb = pers.tile([P, S], F16)
            trib = pers.tile([P, P], F16)
            dummy = pers.tile([1, 1], F16)
            wq_s = w1.tile([P, 8, 512], F16)
            wk_s = w1.tile([P, 8, 512], F16)
            wv_s = w1.tile([P, 8, 512], F16)
            wo_s = wo.tile([P, 4, D], F16)
            xs_tiles = []
            for _sl in range(4):
                xs_t = x1.tile([P, 8, 512], F16, tag="xs")
                xs_tiles.append(xs_t)

            # DMA order: unblock the first q-projection matmul (xs0-kc0 +
            # wq-kc0) in 2 dispatches, then stream the rest in need order.
            nc.sync.dma_start(xs_tiles[0][:, 0:1, :], xT3[:, 0:1, ts(0, 512)])
            nc.sync.dma_start(wq_s[:, 0:1, :], wq3[:, 0:1, :])
            nc.sync.dma_start(xs_tiles[0][:, 1:4, :], xT3[:, 1:4, ts(0, 512)])
            nc.sync.dma_start(wq_s[:, 1:4, :], wq3[:, 1:4, :])
            nc.sync.dma_start(xs_tiles[0][:, 4:8, :], xT3[:, 4:8, ts(0, 512)])
            nc.sync.dma_start(wq_s[:, 4:8, :], wq3[:, 4:8, :])
            for kh in range(2):
                k4 = slice(4 * kh, 4 * kh + 4)
                nc.sync.dma_start(wk_s[:, k4, :], wk3[:, k4, :])
            nc.sync.dma_start(cosb[:], cosT_d.ap())
            nc.sync.dma_start(sinb[:], sinT_d.ap())
            nc.sync.dma_start(trib[:], tri_d.ap())
            for kh in range(2):
                k4 = slice(4 * kh, 4 * kh + 4)
                nc.sync.dma_start(wv_s[:, k4, :], wv3[:, k4, :])
            for sl in (1, 2, 3):
                for kh in range(4):
                    k2 = slice(2 * kh, 2 * kh + 2)
                    nc.sync.dma_start(
                        xs_tiles[sl][:, k2, :], xT3[:, k2, ts(sl, 512)]
                    )
            nc.sync.dma_start(wo_s[:], wo3)
            nc.vector.memset(vA[:, :, :, 64:65], 1.0)
            nc.vector.memset(dummy[:], 0.0)
            # zero the dead halves of the padded k tiles once; the score
            # matmuls then contract over all 128 partitions safely
            nc.vector.memset(kT0p[64:P, :, :], 0.0)
            nc.vector.memset(kT1p[0:64, :, :], 0.0)
            # dummy 1-element exp: forces the ACT Exp table load during the
            # prologue instead of on the first real exp chain
            wrm = rcd.tile([1, 1], F16, tag="wrm")
            nc.scalar.activation(wrm[:], dummy[:], Act.Exp)

            # PSUM: psP (proj/v) 2 banks + psB (sc) 4 + psC (pa) 2 = 8
            _psP_cm = tc.tile_pool(name="psP", bufs=2, space="PSUM")
            _psB_cm = tc.tile_pool(name="psB", bufs=2, space="PSUM")
            _psC_cm = tc.tile_pool(name="psC", bufs=2, space="PSUM")
            psP = _psP_cm.__enter__()
            psB = _psB_cm.__enter__()
            psC = _psC_cm.__enter__()

            def rope(pq, dsts, sls):
                # fp16 on-chip rope. The +-32 partition swap runs as two
                # SBUF->SBUF DMAs (each an affine 2-block partition
                # pattern) on otherwise-idle DMA queues; DVE then does one
                # full-width signed-sin multiply instead of 4 strip-muls.
                pq_s = sq.tile([P, 512], F16, tag="pqs")
                nc.vector.tensor_copy(pq_s[:], pq[:])
                tA = tmp1.tile([P, 512], F16, tag="tA")
                nc.vector.tensor_mul(tA[:], pq_s[:], cosb[:, sls])
                tBs = tmp1.tile([P, 512], F16, tag="tBs")
                for hb, eng in ((0, nc.vector), (64, nc.gpsimd)):
                    eng.tensor_mul(
                        tBs[hb : hb + 32, :],
                        pq_s[hb + 32 : hb + 64, :],
                        sinb[hb + 32 : hb + 64, sls],
                    )
                    eng.tensor_mul(
                        tBs[hb + 32 : hb + 64, :],
                        pq_s[hb : hb + 32, :],
                        sinb[hb : hb + 32, sls],
                    )
                for dst, lo, hi in dsts:
                    nc.vector.tensor_add(dst, tA[lo:hi, :], tBs[lo:hi, :])

            def q_dsts(hc, sls):
                return [(qT[:, hc, sls], 0, P)]

            def k_dsts(hc, sls):
                return [(kT0p[0:64, hc, sls], 0, 64), (kT1p[64:P, hc, sls], 64, P)]

            def proj_one(hc, sl, w_s, dst_fn):
                # one q-or-k projection blob (8 matmuls); rope deferred
                sls = ts(sl, 512)
                xs = xs_tiles[sl]
                pq = psP.tile([P, 512], F32, tag="pp")
                for kc in range(8):
                    nc.tensor.matmul(
                        pq[:], w_s[:, kc, ts(hc, P)], xs[:, kc, :],
                        start=(kc == 0), stop=(kc == 7),
                    )
                return (pq, dst_fn(hc, sls), sls)

            def v_chunk(sl, half):
                # half a v-slice (2 of 4 t4 blocks, 16 matmuls); PSUM
                # evacuation on ScalarE (idle during hc=0's attention)
                xs = xs_tiles[sl]
                for t4 in range(2 * half, 2 * half + 2):
                    pv = psP.tile([P, 512], F32, tag="pp")
                    for kc in range(8):
                        nc.tensor.matmul(
                            pv[:], xs[:, kc, ts(t4, P)], wv_s[:, kc, :],
                            start=(kc == 0), stop=(kc == 7),
                        )
                    nc.scalar.copy(
                        vA[:, sl * 4 + t4, :, 0:64],
                        pv.rearrange("p (h c) -> p h c", h=8),
                    )
                return None

            def sc_mms(hc, j, i):
                # packed causal layout: head0 cols [w0:512] (q -> col q),
                # head1 cols [512:1024-w0] (q -> col 512+q-w0). K=128 via
                # the zero-padded kT tiles: no tiling-mode switch.
                w0 = max(i - 4 * j, 0) * P
                sc = psB.tile([P, 1024], F32, tag="sc")
                nc.tensor.matmul(
                    sc[:, w0:512], kT0p[:, hc, ts(i, P)],
                    qT[:, hc, j * 512 + w0 : (j + 1) * 512],
                    start=True, stop=True,
                )
                nc.tensor.matmul(
                    sc[:, 512 : 1024 - w0], kT1p[:, hc, ts(i, P)],
                    qT[:, hc, j * 512 + w0 : (j + 1) * 512],
                    start=True, stop=True,
                )
                return sc, w0

            def exp_pa(hc, j, i, sc, w0, pa0, pa1, last):
                # one contiguous exp over both heads' causal region; the
                # diagonal 128-blocks are zeroed post-exp by fp16 tri-mask
                # multiplies (the masked region of sc holds stale-but-finite
                # PSUM values, so exp is safe).
                diag = i - 4 * j >= 0
                pt = ptp.tile([P, 1024], F16, tag="pt")
                nc.scalar.activation(
                    pt[:, 0 : 1024 - 2 * w0], sc[:, w0 : 1024 - w0], Act.Exp
                )
                if diag:
                    nc.vector.tensor_mul(pt[:, 0:P], pt[:, 0:P], trib[:])
                    nc.vector.tensor_mul(
                        pt[:, 512 - w0 : 640 - w0],
                        pt[:, 512 - w0 : 640 - w0], trib[:],
                    )
                nc.tensor.matmul(
                    pa0[:, w0:512], vA[:, i, 2 * hc, :],
                    pt[:, 0 : 512 - w0],
                    start=(i == 0), stop=(i == last),
                )
                nc.tensor.matmul(
                    pa1[:, w0:512], vA[:, i, 2 * hc + 1, :],
                    pt[:, 512 - w0 : 1024 - 2 * w0],
                    start=(i == 0), stop=(i == last),
                )

            den_tiles = {}

            def attention_block(hc, j, den_d, rbase, fillers=None):
                # rolling pipeline: sc(i+1) prefetches while exp(i) runs;
                # projection fillers emitted every other i-block so ScalarE's
                # exp stream overlaps projection matmuls
                pa0 = psC.tile([65, 512], F32, tag="pa")
                pa1 = psC.tile([65, 512], F32, tag="pa")
                last = 4 * j + 3
                pending = []
                fb = list(fillers or [])
                sc_prev = sc_mms(hc, j, 0)
                for i in range(last + 1):
                    sc_next = sc_mms(hc, j, i + 1) if i < last else None
                    if i % 2 == 1 and fb:
                        res = fb.pop(0)()
                        if res is not None:
                            pending.append(res)
                    exp_pa(hc, j, i, *sc_prev, pa0, pa1, last)
                    sc_prev = sc_next
                srows = []
                for h01, pa in ((0, pa0), (1, pa1)):
                    srow = rcp.tile([1, 512], F32, tag="srow")
                    nc.vector.tensor_copy(srow[:], pa[64:65, 0:512])
                    if den_d is not None:
                        nc.sync.dma_start(
                            den_d[rbase + h01 : rbase + h01 + 1, :], srow[:]
                        )
                    srows.append(srow)
                    nc.vector.tensor_copy(
                        outT[h01 * 64 : h01 * 64 + 64, hc, ts(j, 512)],
                        pa[0:64, 0:512],
                    )
                for res in pending:
                    rope(*res)
                return srows

            def epilogue_pair(hc):
                den_sb = rcd.tile([8, 512], F32, tag="densb")
                nc.sync.dma_start(den_sb[:], den_tiles[hc][:])
                rec32 = rcd.tile([8, 512], F32, tag="rec32")
                nc.vector.reciprocal_approx_fast(rec32[:], den_sb[:])
                rec8 = rcd.tile([8, 512], F16, tag="rec8")
                with nc.allow_low_precision(reason="fp16 softmax denom"):
                    nc.vector.tensor_copy(rec8[:], rec32[:])
                rec_d = drm.tile([8, 512], F16, tag="recd")
                nc.sync.dma_start(rec_d[:], rec8[:])
                for j in range(4):
                    rb = rbp.tile([P, 512], F16, tag="rb")
                    for h01 in range(2):
                        r = 2 * j + h01
                        nc.sync.dma_start(
                            rb[h01 * 64 : h01 * 64 + 64, :],
                            rec_d[r : r + 1, :].broadcast_to((64, 512)),
                        )
                    nc.vector.tensor_mul(
                        outT[:, hc, ts(j, 512)], outT[:, hc, ts(j, 512)], rb[:]
                    )

            # ---- output projection helper: runs during pair 3's attention,
            # borrowing psP's two banks (no projections remain there); the
            # final group also borrows psC for a 4-deep ping-pong ----
            def p3_group(j, extra_pool=None):
                for sti, st in enumerate(range(4 * j, 4 * j + 4)):
                    pool = extra_pool if (extra_pool and sti % 2) else psP
                    tag = "pa" if pool is extra_pool else "pp"
                    py0 = pool.tile([P, 512], F32, tag=tag)
                    py1 = pool.tile([P, 512], F32, tag=tag)
                    # jc-outer so the two halves share each outT stationary
                    for jc in range(4):
                        nc.tensor.matmul(
                            py0[:], outT[:, jc, ts(st, P)], wo_s[:, jc, 0:512],
                            start=(jc == 0), stop=(jc == 3),
                        )
                        nc.tensor.matmul(
                            py1[:], outT[:, jc, ts(st, P)], wo_s[:, jc, 512:D],
                            start=(jc == 0), stop=(jc == 3),
                        )
                    yo0 = ysb.tile([P, 512], F16, tag="yo0")
                    yo1 = ysb.tile([P, 512], F16, tag="yo1")
                    with nc.allow_low_precision(reason="fp16 partial y"):
                        nc.vector.tensor_copy(yo0[:], py0[:])
                        nc.vector.tensor_copy(yo1[:], py1[:])
                    nc.sync.dma_start(y_ap[ts(st, P), 0:512], yo0[:])
                    nc.sync.dma_start(y_ap[ts(st, P), 512:D], yo1[:])

            def epilogue_j3(j, srows):
                # per-j denominator chain for the last pair: direct
                # reciprocal on the partition-0 srow tiles, one DRAM hop
                # for the partition-broadcast.
                rec_d2 = drm.tile([2, 512], F32, tag="recd2")
                for h01 in range(2):
                    r32 = rcd.tile([1, 512], F32, tag="r32b")
                    nc.vector.reciprocal_approx_fast(r32[:], srows[h01][:])
                    nc.sync.dma_start(rec_d2[h01 : h01 + 1, :], r32[:])
                rb = rbp.tile([P, 512], F32, tag="rb32")
                for h01 in range(2):
                    nc.sync.dma_start(
                        rb[h01 * 64 : h01 * 64 + 64, :],
                        rec_d2[h01 : h01 + 1, :].broadcast_to((64, 512)),
                    )
                nc.vector.tensor_mul(
                    outT[:, 3, ts(j, 512)], outT[:, 3, ts(j, 512)], rb[:]
                )

            # ---- woven schedule ----
            # mini-prologue: q/k/v for slice 0 of pair 0 only; everything
            # else weaves into the attention batches below.
            rope(*proj_one(0, 0, wq_s, q_dsts))
            rope(*proj_one(0, 0, wk_s, k_dsts))
            v_chunk(0, 0)
            v_chunk(0, 1)

            def qf(hc, sl):
                return lambda: proj_one(hc, sl, wq_s, q_dsts)

            def kf(hc, sl):
                return lambda: proj_one(hc, sl, wk_s, k_dsts)

            def vf(sl, half):
                return lambda: v_chunk(sl, half)

            fillers = {
                (0, 0): [qf(0, 1), kf(0, 1)],
                (0, 1): [vf(1, 0), vf(1, 1), qf(0, 2), kf(0, 2)],
                (0, 2): [vf(2, 0), vf(2, 1), qf(0, 3), kf(0, 3),
                         qf(1, 0), kf(1, 0)],
                (0, 3): [vf(3, 0), vf(3, 1), qf(1, 1), kf(1, 1),
                         qf(1, 2), kf(1, 2), qf(1, 3), kf(1, 3)],
                (1, 0): [qf(2, 0), kf(2, 0)],
                (1, 1): [qf(2, 1), kf(2, 1)],
                (1, 2): [qf(2, 2), kf(2, 2)],
                (1, 3): [qf(2, 3), kf(2, 3)],
                (2, 0): [qf(3, 0), kf(3, 0)],
                (2, 1): [qf(3, 1), kf(3, 1)],
                (2, 2): [qf(3, 2), kf(3, 2)],
                (2, 3): [qf(3, 3), kf(3, 3)],
            }

            for hc in range(3):
                den_d = drm.tile([8, 512], F32, tag="dend")
                den_tiles[hc] = den_d
                for j in range(4):
                    attention_block(hc, j, den_d, 2 * j,
                                    fillers=fillers[(hc, j)])
                if hc >= 1:
                    epilogue_pair(hc - 1)
            # pair 3: j in [3,2,1,0] so the tail attention block is the
            # smallest; p3_group(j) issues after the NEXT attention block so
            # its denominator chain hides under it.
            j_order = [3, 2, 1, 0]
            prev_j = None
            for idx, j in enumerate(j_order):
                srows = attention_block(3, j, None, 0)
                if idx == 0:
                    epilogue_pair(2)
                epilogue_j3(j, srows)
                if prev_j is not None:
                    p3_group(prev_j)
                prev_j = j
            p3_group(prev_j, extra_pool=psC)

            _psC_cm.__exit__(None, None, None)
            _psB_cm.__exit__(None, None, None)
            _psP_cm.__exit__(None, None, None)

    nc.compile()
    return nc


def prep_core_inputs(x, token_ids, Wq, Wk, Wv, Wo, core):
    b, half = divmod(core, 2)
    rows = []
    for h in range(half * 8, half * 8 + 8):
        base = h * DH
        rows.extend(base + np.arange(0, DH, 2))
        rows.extend(base + np.arange(1, DH, 2))
    rows = np.asarray(rows)
    cols = np.arange(half * 512, half * 512 + 512)

    f16 = np.float16
    f32 = np.float32
    inv = THETA ** (-np.arange(0, DH, 2, dtype=np.float64) / DH)
    ang = np.asarray(token_ids, dtype=np.float64)[None, :] * inv[:, None]
    cosT = np.tile(np.cos(ang), (4, 1)).astype(f16)
    # signed sin table, source-indexed: the swap-muls read pq_s and sinT at
    # the SOURCE partitions (rows 0:32 = +sin, 32:64 = -sin, tiled)
    sin_block = np.concatenate([np.sin(ang), -np.sin(ang)], axis=0)
    sinT = np.tile(sin_block, (2, 1)).astype(f16)
    tri = (np.arange(P)[:, None] <= np.arange(P)[None, :]).astype(f16)
    return {
        "xT": np.ascontiguousarray(np.asarray(x[b], f32).T.astype(f16)),
        "wqT": np.ascontiguousarray((np.asarray(Wq, f32)[rows] * 0.125).T.astype(f16)),
        "wkT": np.ascontiguousarray(np.asarray(Wk, f32)[rows].T.astype(f16)),
        "wvT": np.ascontiguousarray(np.asarray(Wv, f32)[cols].T.astype(f16)),
        "woT": np.ascontiguousarray(np.asarray(Wo, f32)[:, cols].T.astype(f16)),
        "cosT": cosT,
        "sinT": sinT,
        "tri": tri,
    }


def get_nc():
    if "nc" not in _CACHE:
        _CACHE["nc"] = build_nc()
    return _CACHE["nc"]


def run_cores(in_maps, trace=False):
    from concourse.bass_utils import run_bass_kernel_spmd

    return run_bass_kernel_spmd(
        get_nc(), in_maps, core_ids=list(range(N_CORES)), trace=trace
    )


def combine(res):
    y = np.empty((B, S, D), np.float32)
    for b in range(B):
        y[b] = res.results[2 * b]["y"].astype(np.float32) + res.results[
            2 * b + 1
        ]["y"].astype(np.float32)
    return y


def kernel(x, token_ids, Wq, Wk, Wv, Wo):
    in_maps = [
        prep_core_inputs(x, token_ids, Wq, Wk, Wv, Wo, c) for c in range(N_CORES)
    ]
    res = run_cores(in_maps)
    return combine(res)


# revision 18
# speedup vs baseline: 1.2188x; 1.0753x over previous
"""Causal multi-head self-attention (B=4, S=2048, D=1024, H=16) on 8 TRN2
NeuronCores.

Sharding: core c = (batch b = c//2, head-half = c%2). Each core computes, for
its batch and its 8 heads: QKV projections (+RoPE), causal softmax attention,
and a row-sharded output projection. The host sums the two partial y's per
batch.

v7 design (on top of v5's fp16 + packed-causal + woven schedule):
  - Zero tiling-mode switches: kT is stored as TWO zero-padded tiles
    (kT0p: head0 dims in partitions 0:64, zeros in 64:128; kT1p mirrored),
    so the score matmuls contract over K=128 and every matmul in the kernel
    runs in the default 128x128 array mode. v5/v6's K=64 row-tiled score
    matmuls overlapped per-pair but paid a PE drain (~110ns) on every
    64<->128 mode transition (~300 of them, ~35us).
  - RoPE: the +-32 partition swap runs as two SBUF->SBUF DMAs (affine
    2-block partition patterns) on otherwise-idle DMA queues, replacing 4
    DVE strip-muls; the sin multiply is then one full-width fp16 op.
  - v-slice PSUM evacuations moved to ScalarE (idle during hc=0's
    attention); projection fillers distributed per-2-i-blocks; pair-3 runs
    j in [3,2,1,0]; final wo group ping-pongs over 4 PSUM banks.
  - Startup: first x/wq chunks DMA'd first so the first matmul issues at
    ~10us (the ~7us engine-barrier preamble + DMA dispatch rate bound it).
"""

import numpy as np

B, S, D = 4, 2048, 1024
NUM_HEADS = 16
THETA = 10000.0
DH = 64
N_CORES = 8
P = 128

_CACHE = {}


def build_nc():
    """Build the single-core SPMD Bass program (identical on all 8 cores)."""
    import concourse.mybir as mybir
    import concourse.tile as tile
    from concourse import bacc
    from concourse.bass import ts

    F16 = mybir.dt.float16
    F32 = mybir.dt.float32
    Act = mybir.ActivationFunctionType

    nc = bacc.Bacc(trn_type="TRN2")
    xT_d = nc.dram_tensor("xT", [D, S], F16, kind="ExternalInput")
    wqT_d = nc.dram_tensor("wqT", [D, 512], F16, kind="ExternalInput")
    wkT_d = nc.dram_tensor("wkT", [D, 512], F16, kind="ExternalInput")
    wvT_d = nc.dram_tensor("wvT", [D, 512], F16, kind="ExternalInput")
    woT_d = nc.dram_tensor("woT", [512, D], F16, kind="ExternalInput")
    cosT_d = nc.dram_tensor("cosT", [P, S], F16, kind="ExternalInput")
    sinT_d = nc.dram_tensor("sinT", [P, S], F16, kind="ExternalInput")
    tri_d = nc.dram_tensor("tri", [P, P], F16, kind="ExternalInput")
    y_d = nc.dram_tensor("y", [S, D], F16, kind="ExternalOutput")

    xT3 = xT_d.ap().rearrange("(kc p) s -> p kc s", p=P)     # [128, 8, 2048]
    wq3 = wqT_d.ap().rearrange("(kc p) j -> p kc j", p=P)    # [128, 8, 512]
    wk3 = wkT_d.ap().rearrange("(kc p) j -> p kc j", p=P)
    wv3 = wvT_d.ap().rearrange("(kc p) j -> p kc j", p=P)
    wo3 = woT_d.ap().rearrange("(jc p) i -> p jc i", p=P)    # [128, 4, 1024]
    y_ap = y_d.ap()

    with tile.TileContext(nc) as tc:
        with (
            tc.tile_pool(name="pers", bufs=1) as pers,
            tc.tile_pool(name="w1", bufs=1) as w1,
            tc.tile_pool(name="x1", bufs=4) as x1,
            tc.tile_pool(name="sq", bufs=4) as sq,
            tc.tile_pool(name="tmp1", bufs=2) as tmp1,
            tc.tile_pool(name="swp", bufs=2) as swp,
            tc.tile_pool(name="ptp", bufs=4) as ptp,
            tc.tile_pool(name="rcp", bufs=4) as rcp,
            tc.tile_pool(name="rcd", bufs=2) as rcd,
            tc.tile_pool(name="rbp", bufs=3) as rbp,
            tc.tile_pool(name="wo", bufs=1) as wo,
            tc.tile_pool(name="ysb", bufs=3) as ysb,
            tc.tile_pool(name="drm", bufs=2, space="DRAM") as drm,
        ):
            qT = pers.tile([P, 4, S], F16)
            kT0p = pers.tile([P, 4, S], F16)
            kT1p = pers.tile([P, 4, S], F16)
            vA = pers.tile([P, 16, 8, 65], F16)
            outT = pers.tile([P, 4, S], F16)
            cosb = pers.tile([P, S], F16)
            sinb = pers.tile([P, S], F16)
            trib = pers.tile([P, P], F16)
            dummy = pers.tile([1, 1], F16)
            wq_s = w1.tile([P, 8, 512], F16)
            wk_s = w1.tile([P, 8, 512], F16)
            wv_s = w1.tile([P, 8, 512], F16)
            wo_s = wo.tile([P, 4, D], F16)
            xs_tiles = []
            for _sl in range(4):
                xs_t = x1.tile([P, 8, 512], F16, tag="xs")
                xs_tiles.append(xs_t)

            # DMA order: unblock the first q-projection matmul (xs0-kc0 +
            # wq-kc0) in 2 dispatches, then stream the rest in need order.
            nc.sync.dma_start(xs_tiles[0][:, 0:1, :], xT3[:, 0:1, ts(0, 512)])
            nc.sync.dma_start(wq_s[:, 0:1, :], wq3[:, 0:1, :])
            nc.sync.dma_start(xs_tiles[0][:, 1:4, :], xT3[:, 1:4, ts(0, 512)])
            nc.sync.dma_start(wq_s[:, 1:4, :], wq3[:, 1:4, :])
            nc.sync.dma_start(xs_tiles[0][:, 4:8, :], xT3[:, 4:8, ts(0, 512)])
            nc.sync.dma_start(wq_s[:, 4:8, :], wq3[:, 4:8, :])
            for kh in range(2):
                k4 = slice(4 * kh, 4 * kh + 4)
                nc.sync.dma_start(wk_s[:, k4, :], wk3[:, k4, :])
            nc.sync.dma_start(cosb[:], cosT_d.ap())
            nc.sync.dma_start(sinb[:], sinT_d.ap())
            nc.sync.dma_start(trib[:], tri_d.ap())
            for kh in range(2):
                k4 = slice(4 * kh, 4 * kh + 4)
                nc.sync.dma_start(wv_s[:, k4, :], wv3[:, k4, :])
            for sl in (1, 2, 3):
                for kh in range(4):
                    k2 = slice(2 * kh, 2 * kh + 2)
                    nc.sync.dma_start(
                        xs_tiles[sl][:, k2, :], xT3[:, k2, ts(sl, 512)]
                    )
            nc.sync.dma_start(wo_s[:], wo3)
            nc.vector.memset(vA[:, :, :, 64:65], 1.0)
            nc.vector.memset(dummy[:], 0.0)
            # zero the dead halves of the padded k tiles once; the score
            # matmuls then contract over all 128 partitions safely
            nc.vector.memset(kT0p[64:P, :, :], 0.0)
            nc.vector.memset(kT1p[0:64, :, :], 0.0)
            # dummy 1-element exp: forces the ACT Exp table load during the
            # prologue instead of on the first real exp chain
            wrm = rcd.tile([1, 1], F16, tag="wrm")
            nc.scalar.activation(wrm[:], dummy[:], Act.Exp)

            # PSUM: psP (proj/v) 2 banks + psB (sc) 4 + psC (pa) 2 = 8
            _psP_cm = tc.tile_pool(name="psP", bufs=2, space="PSUM")
            _psB_cm = tc.tile_pool(name="psB", bufs=2, space="PSUM")
            _psC_cm = tc.tile_pool(name="psC", bufs=2, space="PSUM")
            psP = _psP_cm.__enter__()
            psB = _psB_cm.__enter__()
            psC = _psC_cm.__enter__()

            def rope_copy(pq):
                # evacuate the projection PSUM early so the next filler
                # blob's matmuls get their bank back before the rope math
                pq_s = sq.tile([P, 512], F16, tag="pqs")
                nc.vector.tensor_copy(pq_s[:], pq[:])
                return pq_s

            def rope_math(pq_s, dsts, sls):
                tA = tmp1.tile([P, 512], F16, tag="tA")
                nc.vector.tensor_mul(tA[:], pq_s[:], cosb[:, sls])
                tBs = tmp1.tile([P, 512], F16, tag="tBs")
                for hb in (0, 64):
                    nc.vector.tensor_mul(
                        tBs[hb : hb + 32, :],
                        pq_s[hb + 32 : hb + 64, :],
                        sinb[hb + 32 : hb + 64, sls],
                    )
                    nc.vector.tensor_mul(
                        tBs[hb + 32 : hb + 64, :],
                        pq_s[hb : hb + 32, :],
                        sinb[hb : hb + 32, sls],
                    )
                for dst, lo, hi in dsts:
                    nc.vector.tensor_add(dst, tA[lo:hi, :], tBs[lo:hi, :])

            def q_dsts(hc, sls):
                return [(qT[:, hc, sls], 0, P)]

            def k_dsts(hc, sls):
                return [(kT0p[0:64, hc, sls], 0, 64), (kT1p[64:P, hc, sls], 64, P)]

            def proj_one(hc, sl, w_s, dst_fn):
                # one q-or-k projection blob (8 matmuls); rope deferred
                sls = ts(sl, 512)
                xs = xs_tiles[sl]
                pq = psP.tile([P, 512], F32, tag="pp")
                for kc in range(8):
                    nc.tensor.matmul(
                        pq[:], w_s[:, kc, ts(hc, P)], xs[:, kc, :],
                        start=(kc == 0), stop=(kc == 7),
                    )
                return (pq, dst_fn(hc, sls), sls)

            def v_chunk(sl, half):
                # half a v-slice (2 of 4 t4 blocks, 16 matmuls); PSUM
                # evacuation on ScalarE (idle during hc=0's attention)
                xs = xs_tiles[sl]
                for t4 in range(2 * half, 2 * half + 2):
                    pv = psP.tile([P, 512], F32, tag="pp")
                    for kc in range(8):
                        nc.tensor.matmul(
                            pv[:], xs[:, kc, ts(t4, P)], wv_s[:, kc, :],
                            start=(kc == 0), stop=(kc == 7),
                        )
                    nc.scalar.copy(
                        vA[:, sl * 4 + t4, :, 0:64],
                        pv.rearrange("p (h c) -> p h c", h=8),
                    )
                return None

            def sc_mms(hc, j, i):
                # packed causal layout: head0 cols [w0:512] (q -> col q),
                # head1 cols [512:1024-w0] (q -> col 512+q-w0). K=128 via
                # the zero-padded kT tiles: no tiling-mode switch.
                w0 = max(i - 4 * j, 0) * P
                sc = psB.tile([P, 1024], F32, tag="sc")
                nc.tensor.matmul(
                    sc[:, w0:512], kT0p[:, hc, ts(i, P)],
                    qT[:, hc, j * 512 + w0 : (j + 1) * 512],
                    start=True, stop=True,
                )
                nc.tensor.matmul(
                    sc[:, 512 : 1024 - w0], kT1p[:, hc, ts(i, P)],
                    qT[:, hc, j * 512 + w0 : (j + 1) * 512],
                    start=True, stop=True,
                )
                return sc, w0

            def exp_pa(hc, j, i, sc, w0, pa0, pa1, last):
                # one contiguous exp over both heads' causal region; the
                # diagonal 128-blocks are zeroed post-exp by fp16 tri-mask
                # multiplies (the masked region of sc holds stale-but-finite
                # PSUM values, so exp is safe).
                diag = i - 4 * j >= 0
                pt = ptp.tile([P, 1024], F16, tag="pt")
                nc.scalar.activation(
                    pt[:, 0 : 1024 - 2 * w0], sc[:, w0 : 1024 - w0], Act.Exp
                )
                if diag:
                    # j==0 blocks: the tri-masks land right after the
                    # previous block's rope burst on DVE; route them to the
                    # idle GpSimd queue there so the AV matmuls aren't
                    # stuck behind rope math
                    eng = nc.gpsimd if j == 0 else nc.vector
                    eng.tensor_mul(pt[:, 0:P], pt[:, 0:P], trib[:])
                    eng.tensor_mul(
                        pt[:, 512 - w0 : 640 - w0],
                        pt[:, 512 - w0 : 640 - w0], trib[:],
                    )
                nc.tensor.matmul(
                    pa0[:, w0:512], vA[:, i, 2 * hc, :],
                    pt[:, 0 : 512 - w0],
                    start=(i == 0), stop=(i == last),
                )
                nc.tensor.matmul(
                    pa1[:, w0:512], vA[:, i, 2 * hc + 1, :],
                    pt[:, 512 - w0 : 1024 - 2 * w0],
                    start=(i == 0), stop=(i == last),
                )

            den_tiles = {}

            def attention_block(hc, j, den_d, rbase, fillers=None):
                # rolling pipeline: sc(i+1) prefetches while exp(i) runs;
                # projection fillers emitted every other i-block so ScalarE's
                # exp stream overlaps projection matmuls
                pa0 = psC.tile([65, 512], F32, tag="pa")
                pa1 = psC.tile([65, 512], F32, tag="pa")
                last = 4 * j + 3
                pending = []
                fb = list(fillers or [])
                sc_prev = sc_mms(hc, j, 0)
                for i in range(last + 1):
                    sc_next = sc_mms(hc, j, i + 1) if i < last else None
                    if i % 2 == 1 and fb:
                        res = fb.pop(0)()
                        if res is not None:
                            pending.append(res)
                    exp_pa(hc, j, i, *sc_prev, pa0, pa1, last)
                    sc_prev = sc_next
                srows = []
                for h01, pa in ((0, pa0), (1, pa1)):
                    srow = rcp.tile([1, 512], F32, tag="srow")
                    nc.vector.tensor_copy(srow[:], pa[64:65, 0:512])
                    if den_d is not None:
                        nc.sync.dma_start(
                            den_d[rbase + h01 : rbase + h01 + 1, :], srow[:]
                        )
                    srows.append(srow)
                    nc.vector.tensor_copy(
                        outT[h01 * 64 : h01 * 64 + 64, hc, ts(j, 512)],
                        pa[0:64, 0:512],
                    )
                copied = [(rope_copy(pq), dsts, sls) for pq, dsts, sls in pending]
                for pq_s, dsts, sls in copied:
                    rope_math(pq_s, dsts, sls)
                return srows

            def epilogue_pair(hc):
                den_sb = rcd.tile([8, 512], F32, tag="densb")
                nc.sync.dma_start(den_sb[:], den_tiles[hc][:])
                rec32 = rcd.tile([8, 512], F32, tag="rec32")
                nc.vector.reciprocal_approx_fast(rec32[:], den_sb[:])
                rec8 = rcd.tile([8, 512], F16, tag="rec8")
                with nc.allow_low_precision(reason="fp16 softmax denom"):
                    nc.vector.tensor_copy(rec8[:], rec32[:])
                rec_d = drm.tile([8, 512], F16, tag="recd")
                nc.sync.dma_start(rec_d[:], rec8[:])
                for j in range(4):
                    rb = rbp.tile([P, 512], F16, tag="rb")
                    for h01 in range(2):
                        r = 2 * j + h01
                        nc.sync.dma_start(
                            rb[h01 * 64 : h01 * 64 + 64, :],
                            rec_d[r : r + 1, :].broadcast_to((64, 512)),
                        )
                    nc.vector.tensor_mul(
                        outT[:, hc, ts(j, 512)], outT[:, hc, ts(j, 512)], rb[:]
                    )

            # ---- output projection helper: runs during pair 3's attention,
            # borrowing psP's two banks (no projections remain there); the
            # final group also borrows psC for a 4-deep ping-pong ----
            def p3_group(j, extra_pool=None):
                for sti, st in enumerate(range(4 * j, 4 * j + 4)):
                    pool = extra_pool if (extra_pool and sti % 2) else psP
                    tag = "pa" if pool is extra_pool else "pp"
                    py0 = pool.tile([P, 512], F32, tag=tag)
                    py1 = pool.tile([P, 512], F32, tag=tag)
                    # jc-outer so the two halves share each outT stationary
                    for jc in range(4):
                        nc.tensor.matmul(
                            py0[:], outT[:, jc, ts(st, P)], wo_s[:, jc, 0:512],
                            start=(jc == 0), stop=(jc == 3),
                        )
                        nc.tensor.matmul(
                            py1[:], outT[:, jc, ts(st, P)], wo_s[:, jc, 512:D],
                            start=(jc == 0), stop=(jc == 3),
                        )
                    yo0 = ysb.tile([P, 512], F16, tag="yo0")
                    yo1 = ysb.tile([P, 512], F16, tag="yo1")
                    with nc.allow_low_precision(reason="fp16 partial y"):
                        nc.vector.tensor_copy(yo0[:], py0[:])
                        nc.vector.tensor_copy(yo1[:], py1[:])
                    nc.sync.dma_start(y_ap[ts(st, P), 0:512], yo0[:])
                    nc.sync.dma_start(y_ap[ts(st, P), 512:D], yo1[:])

            def epilogue_j3(j, srows):
                # per-j denominator chain for the last pair: direct
                # reciprocal on the partition-0 srow tiles, one DRAM hop
                # for the partition-broadcast.
                rec_d2 = drm.tile([2, 512], F32, tag="recd2")
                for h01 in range(2):
                    r32 = rcd.tile([1, 512], F32, tag="r32b")
                    nc.vector.reciprocal_approx_fast(r32[:], srows[h01][:])
                    nc.sync.dma_start(rec_d2[h01 : h01 + 1, :], r32[:])
                rb = rbp.tile([P, 512], F32, tag="rb32")
                for h01 in range(2):
                    nc.sync.dma_start(
                        rb[h01 * 64 : h01 * 64 + 64, :],
                        rec_d2[h01 : h01 + 1, :].broadcast_to((64, 512)),
                    )
                nc.vector.tensor_mul(
                    outT[:, 3, ts(j, 512)], outT[:, 3, ts(j, 512)], rb[:]
                )

            # ---- woven schedule ----
            # mini-prologue: q/k/v for slice 0 of pair 0 only; everything
            # else weaves into the attention batches below.
            for pq, dsts, sls in (
                proj_one(0, 0, wq_s, q_dsts),
                proj_one(0, 0, wk_s, k_dsts),
            ):
                rope_math(rope_copy(pq), dsts, sls)
            v_chunk(0, 0)
            v_chunk(0, 1)

            def qf(hc, sl):
                return lambda: proj_one(hc, sl, wq_s, q_dsts)

            def kf(hc, sl):
                return lambda: proj_one(hc, sl, wk_s, k_dsts)

            def vf(sl, half):
                return lambda: v_chunk(sl, half)

            fillers = {
                (0, 0): [qf(0, 1), kf(0, 1)],
                (0, 1): [vf(1, 0), vf(1, 1), qf(0, 2), kf(0, 2)],
                (0, 2): [vf(2, 0), vf(2, 1), qf(0, 3), kf(0, 3),
                         qf(1, 0), kf(1, 0)],
                (0, 3): [vf(3, 0), vf(3, 1), qf(1, 1), kf(1, 1),
                         qf(1, 2), kf(1, 2), qf(1, 3), kf(1, 3)],
                (1, 0): [qf(2, 0), kf(2, 0)],
                (1, 1): [qf(2, 1), kf(2, 1)],
                (1, 2): [qf(2, 2), kf(2, 2)],
                (1, 3): [qf(2, 3), kf(2, 3)],
                (2, 0): [qf(3, 0), kf(3, 0)],
                (2, 1): [qf(3, 1), kf(3, 1)],
                (2, 2): [qf(3, 2), kf(3, 2)],
                (2, 3): [qf(3, 3), kf(3, 3)],
            }

            for hc in range(3):
                den_d = drm.tile([8, 512], F32, tag="dend")
                den_tiles[hc] = den_d
                for j in range(4):
                    attention_block(hc, j, den_d, 2 * j,
                                    fillers=fillers[(hc, j)])
                if hc >= 1:
                    epilogue_pair(hc - 1)
            # pair 3: j in [3,2,1,0] so the tail attention block is the
            # smallest; p3_group(j) issues after the NEXT attention block so
            # its denominator chain hides under it.
            j_order = [3, 2, 1, 0]
            prev_j = None
            for idx, j in enumerate(j_order):
                srows = attention_block(3, j, None, 0)
                if idx == 0:
                    epilogue_pair(2)
                epilogue_j3(j, srows)
                if prev_j is not None:
                    p3_group(prev_j)
                prev_j = j
            p3_group(prev_j, extra_pool=psC)

            _psC_cm.__exit__(None, None, None)
            _psB_cm.__exit__(None, None, None)
            _psP_cm.__exit__(None, None, None)

    nc.compile()
    return nc


def prep_core_inputs(x, token_ids, Wq, Wk, Wv, Wo, core):
    b, half = divmod(core, 2)
    rows = []
    for h in range(half * 8, half * 8 + 8):
        base = h * DH
        rows.extend(base + np.arange(0, DH, 2))
        rows.extend(base + np.arange(1, DH, 2))
    rows = np.asarray(rows)
    cols = np.arange(half * 512, half * 512 + 512)

    f16 = np.float16
    f32 = np.float32
    inv = THETA ** (-np.arange(0, DH, 2, dtype=np.float64) / DH)
    ang = np.asarray(token_ids, dtype=np.float64)[None, :] * inv[:, None]
    cosT = np.tile(np.cos(ang), (4, 1)).astype(f16)
    # signed sin table, source-indexed: the swap-muls read pq_s and sinT at
    # the SOURCE partitions (rows 0:32 = +sin, 32:64 = -sin, tiled)
    sin_block = np.concatenate([np.sin(ang), -np.sin(ang)], axis=0)
    sinT = np.tile(sin_block, (2, 1)).astype(f16)
    tri = (np.arange(P)[:, None] <= np.arange(P)[None, :]).astype(f16)
    return {
        "xT": np.ascontiguousarray(np.asarray(x[b], f32).T.astype(f16)),
        "wqT": np.ascontiguousarray((np.asarray(Wq, f32)[rows] * 0.125).T.astype(f16)),
        "wkT": np.ascontiguousarray(np.asarray(Wk, f32)[rows].T.astype(f16)),
        "wvT": np.ascontiguousarray(np.asarray(Wv, f32)[cols].T.astype(f16)),
        "woT": np.ascontiguousarray(np.asarray(Wo, f32)[:, cols].T.astype(f16)),
        "cosT": cosT,
        "sinT": sinT,
        "tri": tri,
    }


def get_nc():
    if "nc" not in _CACHE:
        _CACHE["nc"] = build_nc()
    return _CACHE["nc"]


def run_cores(in_maps, trace=False):
    from concourse.bass_utils import run_bass_kernel_spmd

    return run_bass_kernel_spmd(
        get_nc(), in_maps, core_ids=list(range(N_CORES)), trace=trace
    )


def combine(res):
    y = np.empty((B, S, D), np.float32)
    for b in range(B):
        y[b] = res.results[2 * b]["y"].astype(np.float32) + res.results[
            2 * b + 1
        ]["y"].astype(np.float32)
    return y


def kernel(x, token_ids, Wq, Wk, Wv, Wo):
    in_maps = [
        prep_core_inputs(x, token_ids, Wq, Wk, Wv, Wo, c) for c in range(N_CORES)
    ]
    res = run_cores(in_maps)
    return combine(res)
